# revision 19
# baseline (speedup 1.0000x reference)
"""Trainium2 Bass kernel for Atom2Bond GNN message passing (forward).

Computation: out[e, :] = relu(concat(atom[src_idx[e]], edge[e]) @ W + b)
  atom_embedding [10000, 128] f32, edge_embedding [640000, 64] f32,
  src_idx [640000] int, W [192, 128] f32, b [128] f32 -> out [640000, 128] f32

Strategy (8 NeuronCores, edges sharded 80000/core, padded to 81920):

  Host-side, per core, edges are SORTED by src_idx.  A 512-edge tile's
  (sorted) source atoms span < 80 consecutive ids [lo, lo+80).  The
  gather + concat + matmul + bias collapses into ONE full-array K=128
  mixed-precision matmul per tile plus a tiny corrective matmul:

    moving  [128, 512] fp8(e3m4): rows 0-63  = one-hot H_low[a, e]
                                    (1 iff src[e] == lo+a, a < 64)
                                  rows 64-127 = edge features (transposed)
    station [128, 128] fp16:      rows 0-63  = P[lo .. lo+64)
                                    where P = atom @ Wa + b (bias folded)
                                  rows 64-127 = We (constant, pre-filled
                                    once per SBUF buffer)

  Edges whose source lands in [lo+64, lo+80) (a sorted SUFFIX of <= 64
  edges, max 64 measured on this input) are covered by a K=16, 64-col
  accumulate matmul from a separate one-hot overflow stream.  The
  one-hot H is exact 0/1 in fp8; fp16-stationary x fp8-moving matmuls
  were verified exact on HW.

  The K=128 main matmuls keep the PE's HAM activity monitor in the
  un-throttled 8/8 clock state (1.2 GHz -> 2.4 GHz): HAM only counts
  full-array activity, so a schedule of K<=80 matmuls that ever goes
  idle for one 3.4us window is stuck at half clock for the rest of the
  run (measured: 427ns vs 213ns per 512-col matmul).

  Two 512-edge tiles (one 2-bank PSUM tile) per pipeline step, 4 tagged
  PSUM buffers; the pure-ReLU epilogue converts f32 PSUM directly to
  fp8 SBUF alternating scalar/vector so adjacent steps drain
  concurrently; half-chunk fp8 drains ride the GpSimd SWDGE queue.
  All input DMAs are sliced to <= 4 KB per partition (measured SDMA
  packet rates: 4 KB 24.9 GB/s/engine vs 16 KB 14.4).  Output is
  written transposed in sorted-edge order; the host un-sorts.

  Per-core HBM traffic: moving 10.5 MB + P windows 3.3 MB + overflow
  0.8 MB + out 10.5 MB = ~25 MB.  e3m4 holds H exactly, edge features
  and relu outputs (|x| <= ~5.5 < 15.5 max) with ~0.4-1.3% RMS
  rounding; measured end-to-end rel err 1.55e-2 (gate 2e-2).
"""

import numpy as np
import ml_dtypes

F16 = np.float16
E3 = ml_dtypes.float8_e3m4

N_NODES = 10000
N_EDGES = 640000
NODE_DIM = 128
EDGE_DIM = 64
N_CORES = 8

EPC = N_EDGES // N_CORES          # 80000 edges per core
TILE = 512                        # edges per matmul tile
CHUNK = 16384                     # edges per pipeline chunk (32 tiles)
TPC = CHUNK // TILE               # 32 tiles per chunk
EPAD = 81920                      # EPC padded to a multiple of CHUNK
NCHUNK = EPAD // CHUNK            # 5
NTILE = EPAD // TILE              # 160 tiles per core
KLOW = 64                         # atoms in the main window
KOVF = 16                         # atoms in the overflow window
SOVF = 64                         # overflow edge suffix length per tile
KROWS = KLOW + KOVF               # 80: max span must stay below this

TRACE = False                     # set True from test.py for NTFF profiling
LAST_RESULTS = None               # BassKernelResults of last run

_NC = None                        # cached compiled Bacc module


def _build_module():
    from contextlib import ExitStack

    import concourse.bacc as bacc
    import concourse.mybir as mybir
    import concourse.tile as tile

    nc = bacc.Bacc("TRN2", target_bir_lowering=False, debug=False)

    # Per-chunk-major host layouts so every chunk DMA is fully contiguous.
    mv = nc.dram_tensor(
        "mv", [NCHUNK, 128, TPC * TILE], mybir.dt.float8e3, kind="ExternalInput"
    )
    plo = nc.dram_tensor(
        "plo", [NCHUNK, KLOW, TPC * 128], mybir.dt.float16, kind="ExternalInput"
    )
    pov = nc.dram_tensor(
        "pov", [NCHUNK, KOVF, TPC * 128], mybir.dt.float16, kind="ExternalInput"
    )
    hov = nc.dram_tensor(
        "hov", [NCHUNK, KOVF, TPC * SOVF], mybir.dt.float8e3, kind="ExternalInput"
    )
    werep = nc.dram_tensor(
        "werep", [EDGE_DIM, TPC * 128], mybir.dt.float16, kind="ExternalInput"
    )
    outt = nc.dram_tensor("outt", [128, EPAD], mybir.dt.float8e3, kind="ExternalOutput")

    with tile.TileContext(nc) as tc, ExitStack() as ctx:
        singles = ctx.enter_context(tc.tile_pool(name="singles", bufs=1))
        pwp = ctx.enter_context(tc.tile_pool(name="pwp", bufs=1))
        mvp = ctx.enter_context(tc.tile_pool(name="mvp", bufs=4))
        povp = ctx.enter_context(tc.tile_pool(name="povp", bufs=3))
        hovp = ctx.enter_context(tc.tile_pool(name="hovp", bufs=3))
        outp = ctx.enter_context(tc.tile_pool(name="outp", bufs=3))
        psump = ctx.enter_context(tc.tile_pool(name="psum", bufs=1, space="PSUM"))

        # Dummy matmuls bridge the PE from t~6us until the first chunk's
        # data lands: the HAM clock gate re-throttles the PE to 4/8
        # (1.2 GHz) after any fully-idle 3.4us window, and only K=128
        # full-array matmuls re-arm it.  The warmup source is a memset
        # tile, NOT a DMA'd one — it must not wait on any load.  The
        # K=128 main matmuls below keep HAM armed from then on.
        wsrc = singles.tile([128, 128], mybir.dt.float16)
        nc.vector.memset(wsrc[:], 1.0)
        warm = psump.tile([128, 2 * TILE], mybir.dt.float32, tag="ps0")
        for _ in range(90):
            nc.tensor.matmul(
                warm[:, 0:128], wsrc[:], wsrc[:], start=True, stop=True
            )

        # stationary buffers: rows 0-63 reloaded per chunk (P window),
        # rows 64-127 = We, filled once per buffer here (werep rides the
        # scalar queue; sliced to 4KB per partition)
        pw_sbs = []
        for i in range(3):
            pw_sb = pwp.tile(
                [128, TPC, 128], mybir.dt.float16, tag=f"pw{i}", name=f"pw{i}"
            )
            for q in range(2):
                sl = slice(q * (TPC // 2), (q + 1) * (TPC // 2))
                nc.scalar.dma_start(
                    out=pw_sb[EDGE_DIM:128, sl, :],
                    in_=werep.rearrange("a (t f) -> a t f", t=TPC)[:, sl, :],
                )
            pw_sbs.append(pw_sb)

        for c in range(NCHUNK):
            # inputs sliced to 4KB/partition and SPLIT across both HWDGE
            # rings (sync + scalar): a single ring's HBM-read packets
            # pipeline at only ~17 GB/s/engine (writes hit ~24), so one
            # ring alone starves the PE into HAM re-throttle.  Chunk 0's
            # pw/pov also ride the SWDGE ring to land extra early.
            mv_sb = mvp.tile([128, TPC, TILE], mybir.dt.float8e3)
            for q in range(4):
                sl = slice(q * (TPC // 4), (q + 1) * (TPC // 4))
                (nc.scalar if q % 2 else nc.sync).dma_start(
                    out=mv_sb[:, sl, :],
                    in_=mv[c].rearrange("a (t f) -> a t f", t=TPC)[:, sl, :],
                )
            pw_sb = pw_sbs[c % 3]
            for q in range(2):
                sl = slice(q * (TPC // 2), (q + 1) * (TPC // 2))
                eng = nc.gpsimd if c == 0 else (nc.scalar if q else nc.sync)
                eng.dma_start(
                    out=pw_sb[0:KLOW, sl, :],
                    in_=plo[c].rearrange("a (t f) -> a t f", t=TPC)[:, sl, :],
                )
            pov_sb = povp.tile([KOVF, TPC, 128], mybir.dt.float16)
            (nc.gpsimd if c == 0 else nc.sync).dma_start(
                out=pov_sb[:], in_=pov[c].rearrange("a (t f) -> a t f", t=TPC)
            )
            hov_sb = hovp.tile([KOVF, TPC, SOVF], mybir.dt.float8e3)
            nc.scalar.dma_start(
                out=hov_sb[:], in_=hov[c].rearrange("a (t f) -> a t f", t=TPC)
            )

            out_sb = outp.tile([128, CHUNK], mybir.dt.float8e3)
            # one 2-tile step per 2-bank PSUM buffer; 4 tagged buffers
            for m in range(TPC // 2):
                step = c * (TPC // 2) + m
                ps = psump.tile(
                    [128, 2 * TILE], mybir.dt.float32, tag=f"ps{step % 4}", name="ps"
                )
                for k in range(2):
                    j = 2 * m + k
                    nc.tensor.matmul(
                        ps[:, k * TILE : (k + 1) * TILE],
                        pw_sb[:, j, :],
                        mv_sb[:, j, :],
                        start=True,
                        stop=False,
                    )
                for k in range(2):
                    j = 2 * m + k
                    nc.tensor.matmul(
                        ps[:, (k + 1) * TILE - SOVF : (k + 1) * TILE],
                        pov_sb[:, j, :],
                        hov_sb[:, j, :],
                        start=False,
                        stop=True,
                    )
                ss = slice(2 * m * TILE, (2 * m + 2) * TILE)
                # pure-ReLU epilogue (bias folded into P host-side), f32
                # PSUM -> fp8 SBUF; strict scalar/vector alternation so
                # adjacent steps' drains overlap on different banks
                if step % 2 == 0:
                    nc.scalar.activation(
                        out_sb[:, ss],
                        ps[:],
                        mybir.ActivationFunctionType.Relu,
                    )
                else:
                    nc.vector.tensor_scalar(
                        out_sb[:, ss],
                        ps[:],
                        0.0,
                        None,
                        mybir.AluOpType.max,
                    )
                # drain half-chunks on the otherwise-idle SWDGE queue
                if m % 8 == 7:
                    ds = slice((2 * m - 14) * TILE, (2 * m + 2) * TILE)
                    nc.gpsimd.dma_start(
                        out=outt[:, c * CHUNK + (2 * m - 14) * TILE : c * CHUNK + (2 * m + 2) * TILE],
                        in_=out_sb[:, ds],
                    )

    nc.compile()
    return nc


def _get_module():
    global _NC
    if _NC is None:
        _NC = _build_module()
    return _NC


def _install_axon_ntff_shim():
    """Register the NTFF profile hook that run_bass_kernel_spmd(trace=True)
    expects under axon; the agent image lacks antenv.axon_hooks."""
    import sys
    import types

    if "antenv.axon_hooks" in sys.modules:
        return
    try:
        from trn_agent_boot.trn_boot import _ntff_profile_via_ctypes

        hook = _ntff_profile_via_ctypes("/opt/axon/libaxon_pjrt.so")
    except Exception:
        hook = None
    mod = types.ModuleType("antenv.axon_hooks")
    mod.get_axon_ntff_profile_hook = lambda: hook
    mod.set_axon_ntff_profile_hook = lambda h: None
    sys.modules["antenv.axon_hooks"] = mod


def _prep_core_inputs(atom_embedding, edge_embedding, src_idx, W, b):
    """Host-side shard + sort + layout prep. Returns (in_maps, orders)."""
    atom_embedding = np.asarray(atom_embedding, dtype=np.float32)
    edge_embedding = np.asarray(edge_embedding, dtype=np.float32)
    src_idx = np.asarray(src_idx).astype(np.int64)
    W = np.asarray(W, dtype=np.float32)
    b = np.asarray(b, dtype=np.float32)

    # P[i] = atom_pad[i] @ Wa + b ; padded so any tile row slice is in range.
    n_pad = N_NODES + KROWS
    atom_pad = np.zeros((n_pad, NODE_DIM), np.float32)
    atom_pad[:N_NODES] = atom_embedding
    P = (atom_pad @ W[:NODE_DIM] + b).astype(F16)       # [n_pad, 128]

    # We broadcast per tile position: werep[p, j, f] = W[128+p, f]
    werep_h = np.ascontiguousarray(
        np.broadcast_to(W[NODE_DIM:, None, :], (EDGE_DIM, TPC, 128))
    ).astype(F16).reshape(EDGE_DIM, TPC * 128)

    alow = np.arange(KLOW, dtype=np.int16)
    aovf = np.arange(KOVF, dtype=np.int16)
    in_maps = []
    orders = []
    for c in range(N_CORES):
        e0 = c * EPC
        idx_core = src_idx[e0 : e0 + EPC]
        order = np.argsort(idx_core, kind="stable")
        orders.append(order)
        sorted_idx = idx_core[order]
        # pad edges reuse the core's max atom id: keeps sort order and
        # keeps the last tile's atom span tight (outputs are discarded)
        sidx = np.full(EPAD, sorted_idx[-1], np.int64)
        sidx[:EPC] = sorted_idx

        tiles = sidx.reshape(NTILE, TILE)
        lo = tiles[:, 0]                            # [NTILE]
        local = (tiles - lo[:, None]).astype(np.int16)      # [NTILE, TILE]
        span = local[:, -1]
        assert span.max() < KROWS, (
            f"tile atom span {span.max()} >= {KROWS}; sorted-tile assumption broken"
        )
        n_ovf = (local >= KLOW).sum(1)
        assert n_ovf.max() <= SOVF, (
            f"overflow edges {n_ovf.max()} > {SOVF}; suffix assumption broken"
        )

        edge_sorted = np.zeros((EPAD, EDGE_DIM), np.float32)
        edge_sorted[:EPC] = edge_embedding[e0 : e0 + EPC][order]

        # moving stream: one-hot H_low over rows 0-63, edge features
        # (transposed within tile) over rows 64-127
        mv_h = np.empty((NTILE, 128, TILE), np.float32)
        mv_h[:, :KLOW, :] = (
            local[:, None, :] == alow[None, :, None]
        )
        mv_h[:, KLOW:, :] = edge_sorted.reshape(NTILE, TILE, EDGE_DIM).transpose(
            0, 2, 1
        )
        mv_h = np.ascontiguousarray(
            mv_h.reshape(NCHUNK, TPC, 128, TILE).transpose(0, 2, 1, 3)
        ).astype(E3).reshape(NCHUNK, 128, TPC * TILE)

        # P windows, chunk-major
        rows_lo = lo[:, None] + alow[None, :].astype(np.int64)   # [NTILE, 64]
        plo_h = np.ascontiguousarray(
            P[rows_lo].reshape(NCHUNK, TPC, KLOW, 128).transpose(0, 2, 1, 3)
        ).reshape(NCHUNK, KLOW, TPC * 128)
        rows_ov = lo[:, None] + KLOW + aovf[None, :].astype(np.int64)
        pov_h = np.ascontiguousarray(
            P[rows_ov].reshape(NCHUNK, TPC, KOVF, 128).transpose(0, 2, 1, 3)
        ).reshape(NCHUNK, KOVF, TPC * 128)

        # overflow one-hot over the last SOVF edges of each tile
        suffix = local[:, TILE - SOVF :]                    # [NTILE, SOVF]
        hov_h = (
            suffix[:, None, :] == (KLOW + aovf)[None, :, None]
        )
        hov_h = np.ascontiguousarray(
            hov_h.reshape(NCHUNK, TPC, KOVF, SOVF).transpose(0, 2, 1, 3)
        ).astype(np.int8).view(np.uint8)
        # 1.0 in e3m4 is 0x30
        hov_h = (hov_h * 0x30).astype(np.uint8).view(E3).reshape(
            NCHUNK, KOVF, TPC * SOVF
        )

        in_maps.append(
            {
                "mv": mv_h,
                "plo": plo_h,
                "pov": pov_h,
                "hov": hov_h,
                "werep": werep_h,
            }
        )
    return in_maps, orders


def kernel(atom_embedding, edge_embedding, src_idx, W, b):
    global LAST_RESULTS
    from concourse.bass_utils import run_bass_kernel_spmd

    nc = _get_module()
    in_maps, orders = _prep_core_inputs(
        atom_embedding, edge_embedding, src_idx, W, b
    )

    kwargs = {}
    if TRACE:
        _install_axon_ntff_shim()
        import concourse.bass_utils as bu

        bu.upload_artifacts = lambda tmpdir: tmpdir  # no bucket in this sandbox
        kwargs = dict(trace=True)

    res = run_bass_kernel_spmd(nc, in_maps, core_ids=list(range(N_CORES)), **kwargs)
    LAST_RESULTS = res

    out = np.empty((N_EDGES, NODE_DIM), np.float32)
    for c in range(N_CORES):
        outt = np.asarray(res.results[c]["outt"])   # [128, EPAD] e3m4
        sorted_out = outt[:, :EPC].T.astype(np.float32)
        out[c * EPC + orders[c]] = sorted_out
    return out


# revision 20
# speedup vs baseline: 1.0542x; 1.0542x over previous
"""Trainium2 Bass kernel for Atom2Bond GNN message passing (forward).

Computation: out[e, :] = relu(concat(atom[src_idx[e]], edge[e]) @ W + b)
  atom_embedding [10000, 128] f32, edge_embedding [640000, 64] f32,
  src_idx [640000] int, W [192, 128] f32, b [128] f32 -> out [640000, 128] f32

Strategy (8 NeuronCores, edges sharded 80000/core, padded to 81920):

  Host-side, per core, edges are SORTED by src_idx.  A 512-edge tile's
  (sorted) source atoms span < 80 consecutive ids [lo, lo+80).  The
  gather + concat + matmul + bias collapses into ONE full-array K=128
  mixed-precision matmul per tile plus a tiny corrective matmul:

    moving  [128, 512] fp8(e3m4): rows 0-63  = one-hot H_low[a, e]
                                    (1 iff src[e] == lo+a, a < 64)
                                  rows 64-127 = edge features (transposed)
    station [128, 128] fp16:      rows 0-63  = P[lo .. lo+64)
                                    where P = atom @ Wa + b (bias folded)
                                  rows 64-127 = We (constant, pre-filled
                                    once per SBUF buffer)

  Edges whose source lands in [lo+64, lo+80) (a sorted SUFFIX of <= 64
  edges, max 64 measured on this input) are covered by a K=16, 64-col
  accumulate matmul from a separate one-hot overflow stream.  The
  one-hot H is exact 0/1 in fp8; fp16-stationary x fp8-moving matmuls
  were verified exact on HW.

  The K=128 main matmuls keep the PE's HAM activity monitor in the
  un-throttled 8/8 clock state (1.2 GHz -> 2.4 GHz): HAM only counts
  full-array activity, so a schedule of K<=80 matmuls that ever goes
  idle for one 3.4us window is stuck at half clock for the rest of the
  run (measured: 427ns vs 213ns per 512-col matmul).

  Two 512-edge tiles (one 2-bank PSUM tile) per pipeline step, 4 tagged
  PSUM buffers; the pure-ReLU epilogue converts f32 PSUM directly to
  fp8 SBUF alternating scalar/vector so adjacent steps drain
  concurrently; half-chunk fp8 drains ride the GpSimd SWDGE queue.
  All input DMAs are sliced to <= 4 KB per partition (measured SDMA
  packet rates: 4 KB 24.9 GB/s/engine vs 16 KB 14.4).  Output is
  written transposed in sorted-edge order; the host un-sorts.

  Per-core HBM traffic: moving 10.5 MB + P windows 3.3 MB + overflow
  0.8 MB + out 10.5 MB = ~25 MB.  e3m4 holds H exactly, edge features
  and relu outputs (|x| <= ~5.5 < 15.5 max) with ~0.4-1.3% RMS
  rounding; measured end-to-end rel err 1.55e-2 (gate 2e-2).
"""

import numpy as np
import ml_dtypes

F16 = np.float16
E3 = ml_dtypes.float8_e3m4

N_NODES = 10000
N_EDGES = 640000
NODE_DIM = 128
EDGE_DIM = 64
N_CORES = 8

EPC = N_EDGES // N_CORES          # 80000 edges per core
TILE = 512                        # edges per matmul tile
CHUNK = 4096                      # edges per pipeline chunk (8 tiles): small
                                  # enough that the PE's per-chunk idle stays
                                  # under one 3.4us HAM window (stays warm)
TPC = CHUNK // TILE               # 8 tiles per chunk
EPAD = 81920                      # EPC padded to a multiple of CHUNK
NCHUNK = EPAD // CHUNK            # 20
NTILE = EPAD // TILE              # 160 tiles per core
KLOW = 64                         # atoms in the main window
KOVF = 16                         # atoms in the overflow window
SOVF = 64                         # overflow edge suffix length per tile
KROWS = KLOW + KOVF               # 80: max span must stay below this

TRACE = False                     # set True from test.py for NTFF profiling
LAST_RESULTS = None               # BassKernelResults of last run

_NC = None                        # cached compiled Bacc module


def _build_module():
    from contextlib import ExitStack

    import concourse.bacc as bacc
    import concourse.mybir as mybir
    import concourse.tile as tile

    nc = bacc.Bacc("TRN2", target_bir_lowering=False, debug=False)

    # Per-chunk-major host layouts so every chunk DMA is fully contiguous.
    mv = nc.dram_tensor(
        "mv", [NCHUNK, 128, TPC * TILE], mybir.dt.float8e3, kind="ExternalInput"
    )
    plo = nc.dram_tensor(
        "plo", [NCHUNK, KLOW, TPC * 128], mybir.dt.float16, kind="ExternalInput"
    )
    pov = nc.dram_tensor(
        "pov", [NCHUNK, KOVF, TPC * 128], mybir.dt.float16, kind="ExternalInput"
    )
    hov = nc.dram_tensor(
        "hov", [NCHUNK, KOVF, TPC * SOVF], mybir.dt.float8e3, kind="ExternalInput"
    )
    werep = nc.dram_tensor(
        "werep", [EDGE_DIM, TPC * 128], mybir.dt.float16, kind="ExternalInput"
    )
    outt = nc.dram_tensor("outt", [128, EPAD], mybir.dt.float8e3, kind="ExternalOutput")

    with tile.TileContext(nc) as tc, ExitStack() as ctx:
        singles = ctx.enter_context(tc.tile_pool(name="singles", bufs=1))
        pwp = ctx.enter_context(tc.tile_pool(name="pwp", bufs=1))
        mvp = ctx.enter_context(tc.tile_pool(name="mvp", bufs=6))
        povp = ctx.enter_context(tc.tile_pool(name="povp", bufs=4))
        hovp = ctx.enter_context(tc.tile_pool(name="hovp", bufs=4))
        outp = ctx.enter_context(tc.tile_pool(name="outp", bufs=3))
        psump = ctx.enter_context(tc.tile_pool(name="psum", bufs=1, space="PSUM"))

        # Dummy matmuls bridge the PE from t~6us until the first chunk's
        # data lands: the HAM clock gate re-throttles the PE to 4/8
        # (1.2 GHz) after any fully-idle 3.4us window, and only K=128
        # full-array matmuls re-arm it.  The warmup source is a memset
        # tile, NOT a DMA'd one — it must not wait on any load.  The
        # K=128 main matmuls below keep HAM armed from then on.
        wsrc = singles.tile([128, 128], mybir.dt.float16)
        nc.vector.memset(wsrc[:], 1.0)
        warm = psump.tile([128, 2 * TILE], mybir.dt.float32, tag="ps0")
        for _ in range(70):
            nc.tensor.matmul(
                warm[:, 0:128], wsrc[:], wsrc[:], start=True, stop=True
            )

        # stationary buffers: rows 0-63 reloaded per chunk (P window),
        # rows 64-127 = We, filled once per buffer here (werep rides the
        # scalar queue; sliced to 4KB per partition)
        pw_sbs = []
        for i in range(3):
            pw_sb = pwp.tile(
                [128, TPC, 128], mybir.dt.float16, tag=f"pw{i}", name=f"pw{i}"
            )
            for q in range(2):
                sl = slice(q * (TPC // 2), (q + 1) * (TPC // 2))
                nc.scalar.dma_start(
                    out=pw_sb[EDGE_DIM:128, sl, :],
                    in_=werep.rearrange("a (t f) -> a t f", t=TPC)[:, sl, :],
                )
            pw_sbs.append(pw_sb)

        for c in range(NCHUNK):
            # inputs sliced to 4KB/partition and SPLIT across both HWDGE
            # rings (sync + scalar): a single ring's HBM-read packets
            # pipeline at only ~17 GB/s/engine (writes hit ~24), so one
            # ring alone starves the PE into HAM re-throttle.  Chunk 0's
            # pw/pov also ride the SWDGE ring to land extra early.
            ring_a = nc.sync if c % 2 else nc.scalar
            ring_b = nc.scalar if c % 2 else nc.sync
            mv_sb = mvp.tile([128, TPC, TILE], mybir.dt.float8e3)
            ring_a.dma_start(
                out=mv_sb[:], in_=mv[c].rearrange("a (t f) -> a t f", t=TPC)
            )
            pw_sb = pw_sbs[c % 3]
            (nc.gpsimd if c == 0 else ring_b).dma_start(
                out=pw_sb[0:KLOW, :, :],
                in_=plo[c].rearrange("a (t f) -> a t f", t=TPC),
            )
            pov_sb = povp.tile([KOVF, TPC, 128], mybir.dt.float16)
            (nc.gpsimd if c == 0 else ring_b).dma_start(
                out=pov_sb[:], in_=pov[c].rearrange("a (t f) -> a t f", t=TPC)
            )
            hov_sb = hovp.tile([KOVF, TPC, SOVF], mybir.dt.float8e3)
            ring_b.dma_start(
                out=hov_sb[:], in_=hov[c].rearrange("a (t f) -> a t f", t=TPC)
            )

            out_sb = outp.tile([128, CHUNK], mybir.dt.float8e3)
            # one 2-tile step per 2-bank PSUM buffer; 4 tagged buffers
            for m in range(TPC // 2):
                step = c * (TPC // 2) + m
                ps = psump.tile(
                    [128, 2 * TILE], mybir.dt.float32, tag=f"ps{step % 4}", name="ps"
                )
                for k in range(2):
                    j = 2 * m + k
                    nc.tensor.matmul(
                        ps[:, k * TILE : (k + 1) * TILE],
                        pw_sb[:, j, :],
                        mv_sb[:, j, :],
                        start=True,
                        stop=False,
                    )
                for k in range(2):
                    j = 2 * m + k
                    nc.tensor.matmul(
                        ps[:, (k + 1) * TILE - SOVF : (k + 1) * TILE],
                        pov_sb[:, j, :],
                        hov_sb[:, j, :],
                        start=False,
                        stop=True,
                    )
                ss = slice(2 * m * TILE, (2 * m + 2) * TILE)
                # pure-ReLU epilogue (bias folded into P host-side), f32
                # PSUM -> fp8 SBUF; strict scalar/vector alternation so
                # adjacent steps' drains overlap on different banks
                if step % 2 == 0:
                    nc.scalar.activation(
                        out_sb[:, ss],
                        ps[:],
                        mybir.ActivationFunctionType.Relu,
                    )
                else:
                    nc.vector.tensor_scalar(
                        out_sb[:, ss],
                        ps[:],
                        0.0,
                        None,
                        mybir.AluOpType.max,
                    )
                # drain whole chunks on the otherwise-idle SWDGE queue
                if m == TPC // 2 - 1:
                    nc.gpsimd.dma_start(
                        out=outt[:, c * CHUNK : (c + 1) * CHUNK],
                        in_=out_sb[:],
                    )

    nc.compile()
    return nc


def _get_module():
    global _NC
    if _NC is None:
        _NC = _build_module()
    return _NC


def _install_axon_ntff_shim():
    """Register the NTFF profile hook that run_bass_kernel_spmd(trace=True)
    expects under axon; the agent image lacks antenv.axon_hooks."""
    import sys
    import types

    if "antenv.axon_hooks" in sys.modules:
        return
    try:
        from trn_agent_boot.trn_boot import _ntff_profile_via_ctypes

        hook = _ntff_profile_via_ctypes("/opt/axon/libaxon_pjrt.so")
    except Exception:
        hook = None
    mod = types.ModuleType("antenv.axon_hooks")
    mod.get_axon_ntff_profile_hook = lambda: hook
    mod.set_axon_ntff_profile_hook = lambda h: None
    sys.modules["antenv.axon_hooks"] = mod


def _prep_core_inputs(atom_embedding, edge_embedding, src_idx, W, b):
    """Host-side shard + sort + layout prep. Returns (in_maps, orders)."""
    atom_embedding = np.asarray(atom_embedding, dtype=np.float32)
    edge_embedding = np.asarray(edge_embedding, dtype=np.float32)
    src_idx = np.asarray(src_idx).astype(np.int64)
    W = np.asarray(W, dtype=np.float32)
    b = np.asarray(b, dtype=np.float32)

    # P[i] = atom_pad[i] @ Wa + b ; padded so any tile row slice is in range.
    n_pad = N_NODES + KROWS
    atom_pad = np.zeros((n_pad, NODE_DIM), np.float32)
    atom_pad[:N_NODES] = atom_embedding
    P = (atom_pad @ W[:NODE_DIM] + b).astype(F16)       # [n_pad, 128]

    # We broadcast per tile position: werep[p, j, f] = W[128+p, f]
    werep_h = np.ascontiguousarray(
        np.broadcast_to(W[NODE_DIM:, None, :], (EDGE_DIM, TPC, 128))
    ).astype(F16).reshape(EDGE_DIM, TPC * 128)

    alow = np.arange(KLOW, dtype=np.int16)
    aovf = np.arange(KOVF, dtype=np.int16)
    in_maps = []
    orders = []
    for c in range(N_CORES):
        e0 = c * EPC
        idx_core = src_idx[e0 : e0 + EPC]
        order = np.argsort(idx_core, kind="stable")
        orders.append(order)
        sorted_idx = idx_core[order]
        # pad edges reuse the core's max atom id: keeps sort order and
        # keeps the last tile's atom span tight (outputs are discarded)
        sidx = np.full(EPAD, sorted_idx[-1], np.int64)
        sidx[:EPC] = sorted_idx

        tiles = sidx.reshape(NTILE, TILE)
        lo = tiles[:, 0]                            # [NTILE]
        local = (tiles - lo[:, None]).astype(np.int16)      # [NTILE, TILE]
        span = local[:, -1]
        assert span.max() < KROWS, (
            f"tile atom span {span.max()} >= {KROWS}; sorted-tile assumption broken"
        )
        n_ovf = (local >= KLOW).sum(1)
        assert n_ovf.max() <= SOVF, (
            f"overflow edges {n_ovf.max()} > {SOVF}; suffix assumption broken"
        )

        edge_sorted = np.zeros((EPAD, EDGE_DIM), np.float32)
        edge_sorted[:EPC] = edge_embedding[e0 : e0 + EPC][order]

        # moving stream: one-hot H_low over rows 0-63, edge features
        # (transposed within tile) over rows 64-127
        mv_h = np.empty((NTILE, 128, TILE), np.float32)
        mv_h[:, :KLOW, :] = (
            local[:, None, :] == alow[None, :, None]
        )
        mv_h[:, KLOW:, :] = edge_sorted.reshape(NTILE, TILE, EDGE_DIM).transpose(
            0, 2, 1
        )
        mv_h = np.ascontiguousarray(
            mv_h.reshape(NCHUNK, TPC, 128, TILE).transpose(0, 2, 1, 3)
        ).astype(E3).reshape(NCHUNK, 128, TPC * TILE)

        # P windows, chunk-major
        rows_lo = lo[:, None] + alow[None, :].astype(np.int64)   # [NTILE, 64]
        plo_h = np.ascontiguousarray(
            P[rows_lo].reshape(NCHUNK, TPC, KLOW, 128).transpose(0, 2, 1, 3)
        ).reshape(NCHUNK, KLOW, TPC * 128)
        rows_ov = lo[:, None] + KLOW + aovf[None, :].astype(np.int64)
        pov_h = np.ascontiguousarray(
            P[rows_ov].reshape(NCHUNK, TPC, KOVF, 128).transpose(0, 2, 1, 3)
        ).reshape(NCHUNK, KOVF, TPC * 128)

        # overflow one-hot over the last SOVF edges of each tile
        suffix = local[:, TILE - SOVF :]                    # [NTILE, SOVF]
        hov_h = (
            suffix[:, None, :] == (KLOW + aovf)[None, :, None]
        )
        hov_h = np.ascontiguousarray(
            hov_h.reshape(NCHUNK, TPC, KOVF, SOVF).transpose(0, 2, 1, 3)
        ).astype(np.int8).view(np.uint8)
        # 1.0 in e3m4 is 0x30
        hov_h = (hov_h * 0x30).astype(np.uint8).view(E3).reshape(
            NCHUNK, KOVF, TPC * SOVF
        )

        in_maps.append(
            {
                "mv": mv_h,
                "plo": plo_h,
                "pov": pov_h,
                "hov": hov_h,
                "werep": werep_h,
            }
        )
    return in_maps, orders


def kernel(atom_embedding, edge_embedding, src_idx, W, b):
    global LAST_RESULTS
    from concourse.bass_utils import run_bass_kernel_spmd

    nc = _get_module()
    in_maps, orders = _prep_core_inputs(
        atom_embedding, edge_embedding, src_idx, W, b
    )

    kwargs = {}
    if TRACE:
        _install_axon_ntff_shim()
        import concourse.bass_utils as bu

        bu.upload_artifacts = lambda tmpdir: tmpdir  # no bucket in this sandbox
        kwargs = dict(trace=True)

    res = run_bass_kernel_spmd(nc, in_maps, core_ids=list(range(N_CORES)), **kwargs)
    LAST_RESULTS = res

    out = np.empty((N_EDGES, NODE_DIM), np.float32)
    for c in range(N_CORES):
        outt = np.asarray(res.results[c]["outt"])   # [128, EPAD] e3m4
        sorted_out = outt[:, :EPC].T.astype(np.float32)
        out[c * EPC + orders[c]] = sorted_out
    return out


# revision 22
# speedup vs baseline: 1.0800x; 1.0244x over previous
"""Trainium2 Bass kernel for Atom2Bond GNN message passing (forward).

Computation: out[e, :] = relu(concat(atom[src_idx[e]], edge[e]) @ W + b)
  atom_embedding [10000, 128] f32, edge_embedding [640000, 64] f32,
  src_idx [640000] int, W [192, 128] f32, b [128] f32 -> out [640000, 128] f32

Strategy (8 NeuronCores, edges sharded 80000/core, padded to 81920):

  Host-side, per core, edges are SORTED by src_idx.  A 512-edge tile's
  (sorted) source atoms span < 80 consecutive ids [lo, lo+80).  The
  gather + concat + matmul + bias collapses into ONE full-array K=128
  mixed-precision matmul per tile plus a tiny corrective matmul:

    moving  [128, 512] fp8(e3m4): rows 0-63  = one-hot H_low[a, e]
                                    (1 iff src[e] == lo+a, a < 64)
                                  rows 64-127 = edge features (transposed)
    station [128, 128] fp16:      rows 0-63  = P[lo .. lo+64)
                                    where P = atom @ Wa + b (bias folded)
                                  rows 64-127 = We (constant, pre-filled
                                    once per SBUF buffer)

  Edges whose source lands in [lo+64, lo+80) (a sorted SUFFIX of <= 64
  edges, max 64 measured on this input) are covered by a K=16, 64-col
  accumulate matmul from a separate one-hot overflow stream.  The
  one-hot H is exact 0/1 in fp8; fp16-stationary x fp8-moving matmuls
  were verified exact on HW.

  The K=128 main matmuls keep the PE's HAM activity monitor in the
  un-throttled 8/8 clock state (1.2 GHz -> 2.4 GHz): HAM only counts
  full-array activity, so a schedule of K<=80 matmuls that ever goes
  idle for one 3.4us window is stuck at half clock for the rest of the
  run (measured: 427ns vs 213ns per 512-col matmul).

  Two 512-edge tiles (one 2-bank PSUM tile) per pipeline step, 4 tagged
  PSUM buffers; the pure-ReLU epilogue converts f32 PSUM directly to
  fp8 SBUF alternating scalar/vector so adjacent steps drain
  concurrently; half-chunk fp8 drains ride the GpSimd SWDGE queue.
  All input DMAs are sliced to <= 4 KB per partition (measured SDMA
  packet rates: 4 KB 24.9 GB/s/engine vs 16 KB 14.4).  Output is
  written transposed in sorted-edge order; the host un-sorts.

  Per-core HBM traffic: moving 10.5 MB + P windows 3.3 MB + overflow
  0.8 MB + out 10.5 MB = ~25 MB.  e3m4 holds H exactly, edge features
  and relu outputs (|x| <= ~5.5 < 15.5 max) with ~0.4-1.3% RMS
  rounding; measured end-to-end rel err 1.55e-2 (gate 2e-2).
"""

import numpy as np
import ml_dtypes

F16 = np.float16
E3 = ml_dtypes.float8_e3m4

N_NODES = 10000
N_EDGES = 640000
NODE_DIM = 128
EDGE_DIM = 64
N_CORES = 8

EPC = N_EDGES // N_CORES          # 80000 edges per core
TILE = 512                        # edges per matmul tile
CHUNK = 4096                      # edges per pipeline chunk (8 tiles): small
                                  # enough that the PE's per-chunk idle stays
                                  # under one 3.4us HAM window (stays warm)
TPC = CHUNK // TILE               # 8 tiles per chunk
EPAD = 81920                      # EPC padded to a multiple of CHUNK
NCHUNK = EPAD // CHUNK            # 20
NTILE = EPAD // TILE              # 160 tiles per core
KLOW = 64                         # atoms in the main window
KOVF = 16                         # atoms in the overflow window
SOVF = 64                         # overflow edge suffix length per tile
KROWS = KLOW + KOVF               # 80: max span must stay below this

TRACE = False                     # set True from test.py for NTFF profiling
LAST_RESULTS = None               # BassKernelResults of last run

_NC = None                        # cached compiled Bacc module


def _build_module():
    from contextlib import ExitStack

    import concourse.bacc as bacc
    import concourse.mybir as mybir
    import concourse.tile as tile

    nc = bacc.Bacc("TRN2", target_bir_lowering=False, debug=False)

    # Per-chunk-major host layouts so every chunk DMA is fully contiguous.
    mv = nc.dram_tensor(
        "mv", [NCHUNK, 128, TPC * TILE], mybir.dt.float8e3, kind="ExternalInput"
    )
    plo = nc.dram_tensor(
        "plo", [NCHUNK, KLOW, TPC * 128], mybir.dt.float16, kind="ExternalInput"
    )
    pov = nc.dram_tensor(
        "pov", [NCHUNK, KOVF, TPC * 128], mybir.dt.float16, kind="ExternalInput"
    )
    hov = nc.dram_tensor(
        "hov", [NCHUNK, KOVF, TPC * SOVF], mybir.dt.float8e3, kind="ExternalInput"
    )
    werep = nc.dram_tensor(
        "werep", [EDGE_DIM, TPC * 128], mybir.dt.float16, kind="ExternalInput"
    )
    outt = nc.dram_tensor("outt", [128, EPAD], mybir.dt.float8e3, kind="ExternalOutput")

    with tile.TileContext(nc) as tc, ExitStack() as ctx:
        singles = ctx.enter_context(tc.tile_pool(name="singles", bufs=1))
        pwp = ctx.enter_context(tc.tile_pool(name="pwp", bufs=1))
        mvp = ctx.enter_context(tc.tile_pool(name="mvp", bufs=6))
        povp = ctx.enter_context(tc.tile_pool(name="povp", bufs=4))
        hovp = ctx.enter_context(tc.tile_pool(name="hovp", bufs=4))
        outp = ctx.enter_context(tc.tile_pool(name="outp", bufs=3))
        psump = ctx.enter_context(tc.tile_pool(name="psum", bufs=1, space="PSUM"))

        # Dummy matmuls bridge the PE from t~6us until the first chunk's
        # data lands: the HAM clock gate re-throttles the PE to 4/8
        # (1.2 GHz) after any fully-idle 3.4us window, and only K=128
        # full-array matmuls re-arm it.  The warmup source is a memset
        # tile, NOT a DMA'd one — it must not wait on any load.  The
        # K=128 main matmuls below keep HAM armed from then on.
        wsrc = singles.tile([128, 128], mybir.dt.float16)
        nc.vector.memset(wsrc[:], 1.0)
        zsrc = singles.tile([128, TILE], mybir.dt.float8e3)
        nc.vector.memset(zsrc[:], 0.0)
        warm = psump.tile([128, 2 * TILE], mybir.dt.float32, tag="ps0")
        for _ in range(70):
            nc.tensor.matmul(
                warm[:, 0:128], wsrc[:], wsrc[:], start=True, stop=True
            )

        # stationary buffers: rows 0-63 reloaded per chunk (P window),
        # rows 64-127 = We, filled once per buffer here (werep rides the
        # scalar queue; sliced to 4KB per partition)
        pw_sbs = []
        for i in range(3):
            pw_sb = pwp.tile(
                [128, TPC, 128], mybir.dt.float16, tag=f"pw{i}", name=f"pw{i}"
            )
            for q in range(2):
                sl = slice(q * (TPC // 2), (q + 1) * (TPC // 2))
                nc.scalar.dma_start(
                    out=pw_sb[EDGE_DIM:128, sl, :],
                    in_=werep.rearrange("a (t f) -> a t f", t=TPC)[:, sl, :],
                )
            pw_sbs.append(pw_sb)

        for c in range(NCHUNK):
            # inputs sliced to 4KB/partition and SPLIT across both HWDGE
            # rings (sync + scalar): a single ring's HBM-read packets
            # pipeline at only ~17 GB/s/engine (writes hit ~24), so one
            # ring alone starves the PE into HAM re-throttle.  Chunk 0's
            # pw/pov also ride the SWDGE ring to land extra early.
            ring_a = nc.sync if c % 2 else nc.scalar
            ring_b = nc.scalar if c % 2 else nc.sync
            mv_sb = mvp.tile([128, TPC, TILE], mybir.dt.float8e3)
            ring_a.dma_start(
                out=mv_sb[:], in_=mv[c].rearrange("a (t f) -> a t f", t=TPC)
            )
            pw_sb = pw_sbs[c % 3]
            (nc.gpsimd if c == 0 else ring_b).dma_start(
                out=pw_sb[0:KLOW, :, :],
                in_=plo[c].rearrange("a (t f) -> a t f", t=TPC),
            )
            pov_sb = povp.tile([KOVF, TPC, 128], mybir.dt.float16)
            (nc.gpsimd if c == 0 else ring_b).dma_start(
                out=pov_sb[:], in_=pov[c].rearrange("a (t f) -> a t f", t=TPC)
            )
            hov_sb = hovp.tile([KOVF, TPC, SOVF], mybir.dt.float8e3)
            ring_b.dma_start(
                out=hov_sb[:], in_=hov[c].rearrange("a (t f) -> a t f", t=TPC)
            )

            out_sb = outp.tile([128, CHUNK], mybir.dt.float8e3)
            # one 2-tile step per 2-bank PSUM buffer; 4 tagged buffers
            for m in range(TPC // 2):
                step = c * (TPC // 2) + m
                ps = psump.tile(
                    [128, 2 * TILE], mybir.dt.float32, tag=f"ps{step % 4}", name="ps"
                )
                # a zero-accumulate full-array dummy heads each step: it
                # has no input dependencies, so during any load stall the
                # PE runs it instead of idling — putting a busy burst in
                # every HAM activity window (a fully-idle 3.4us window
                # would halve the PE clock for ~10-30us).  It writes
                # exact zeros (zero moving operand) that the first real
                # matmul then accumulates onto.
                nc.tensor.matmul(
                    ps[:, 0:TILE],
                    wsrc[:],
                    zsrc[:],
                    start=True,
                    stop=False,
                )
                nc.tensor.matmul(
                    ps[:, 0:TILE],
                    pw_sb[:, 2 * m, :],
                    mv_sb[:, 2 * m, :],
                    start=False,
                    stop=False,
                )
                nc.tensor.matmul(
                    ps[:, TILE : 2 * TILE],
                    pw_sb[:, 2 * m + 1, :],
                    mv_sb[:, 2 * m + 1, :],
                    start=True,
                    stop=False,
                )
                for k in range(2):
                    j = 2 * m + k
                    nc.tensor.matmul(
                        ps[:, (k + 1) * TILE - SOVF : (k + 1) * TILE],
                        pov_sb[:, j, :],
                        hov_sb[:, j, :],
                        start=False,
                        stop=True,
                    )
                ss = slice(2 * m * TILE, (2 * m + 2) * TILE)
                # pure-ReLU epilogue (bias folded into P host-side), f32
                # PSUM -> fp8 SBUF; strict scalar/vector alternation so
                # adjacent steps' drains overlap on different banks
                if step % 2 == 0:
                    nc.scalar.activation(
                        out_sb[:, ss],
                        ps[:],
                        mybir.ActivationFunctionType.Relu,
                    )
                else:
                    nc.vector.tensor_scalar(
                        out_sb[:, ss],
                        ps[:],
                        0.0,
                        None,
                        mybir.AluOpType.max,
                    )
                # drain whole chunks on the otherwise-idle SWDGE queue
                if m == TPC // 2 - 1:
                    nc.gpsimd.dma_start(
                        out=outt[:, c * CHUNK : (c + 1) * CHUNK],
                        in_=out_sb[:],
                    )

    nc.compile()
    return nc


def _get_module():
    global _NC
    if _NC is None:
        _NC = _build_module()
    return _NC


def _install_axon_ntff_shim():
    """Register the NTFF profile hook that run_bass_kernel_spmd(trace=True)
    expects under axon; the agent image lacks antenv.axon_hooks."""
    import sys
    import types

    if "antenv.axon_hooks" in sys.modules:
        return
    try:
        from trn_agent_boot.trn_boot import _ntff_profile_via_ctypes

        hook = _ntff_profile_via_ctypes("/opt/axon/libaxon_pjrt.so")
    except Exception:
        hook = None
    mod = types.ModuleType("antenv.axon_hooks")
    mod.get_axon_ntff_profile_hook = lambda: hook
    mod.set_axon_ntff_profile_hook = lambda h: None
    sys.modules["antenv.axon_hooks"] = mod


def _prep_core_inputs(atom_embedding, edge_embedding, src_idx, W, b):
    """Host-side shard + sort + layout prep. Returns (in_maps, orders)."""
    atom_embedding = np.asarray(atom_embedding, dtype=np.float32)
    edge_embedding = np.asarray(edge_embedding, dtype=np.float32)
    src_idx = np.asarray(src_idx).astype(np.int64)
    W = np.asarray(W, dtype=np.float32)
    b = np.asarray(b, dtype=np.float32)

    # P[i] = atom_pad[i] @ Wa + b ; padded so any tile row slice is in range.
    n_pad = N_NODES + KROWS
    atom_pad = np.zeros((n_pad, NODE_DIM), np.float32)
    atom_pad[:N_NODES] = atom_embedding
    P = (atom_pad @ W[:NODE_DIM] + b).astype(F16)       # [n_pad, 128]

    # We broadcast per tile position: werep[p, j, f] = W[128+p, f]
    werep_h = np.ascontiguousarray(
        np.broadcast_to(W[NODE_DIM:, None, :], (EDGE_DIM, TPC, 128))
    ).astype(F16).reshape(EDGE_DIM, TPC * 128)

    alow = np.arange(KLOW, dtype=np.int16)
    aovf = np.arange(KOVF, dtype=np.int16)
    in_maps = []
    orders = []
    for c in range(N_CORES):
        e0 = c * EPC
        idx_core = src_idx[e0 : e0 + EPC]
        order = np.argsort(idx_core, kind="stable")
        orders.append(order)
        sorted_idx = idx_core[order]
        # pad edges reuse the core's max atom id: keeps sort order and
        # keeps the last tile's atom span tight (outputs are discarded)
        sidx = np.full(EPAD, sorted_idx[-1], np.int64)
        sidx[:EPC] = sorted_idx

        tiles = sidx.reshape(NTILE, TILE)
        lo = tiles[:, 0]                            # [NTILE]
        local = (tiles - lo[:, None]).astype(np.int16)      # [NTILE, TILE]
        span = local[:, -1]
        assert span.max() < KROWS, (
            f"tile atom span {span.max()} >= {KROWS}; sorted-tile assumption broken"
        )
        n_ovf = (local >= KLOW).sum(1)
        assert n_ovf.max() <= SOVF, (
            f"overflow edges {n_ovf.max()} > {SOVF}; suffix assumption broken"
        )

        edge_sorted = np.zeros((EPAD, EDGE_DIM), np.float32)
        edge_sorted[:EPC] = edge_embedding[e0 : e0 + EPC][order]

        # moving stream: one-hot H_low over rows 0-63, edge features
        # (transposed within tile) over rows 64-127
        mv_h = np.empty((NTILE, 128, TILE), np.float32)
        mv_h[:, :KLOW, :] = (
            local[:, None, :] == alow[None, :, None]
        )
        mv_h[:, KLOW:, :] = edge_sorted.reshape(NTILE, TILE, EDGE_DIM).transpose(
            0, 2, 1
        )
        mv_h = np.ascontiguousarray(
            mv_h.reshape(NCHUNK, TPC, 128, TILE).transpose(0, 2, 1, 3)
        ).astype(E3).reshape(NCHUNK, 128, TPC * TILE)

        # P windows, chunk-major
        rows_lo = lo[:, None] + alow[None, :].astype(np.int64)   # [NTILE, 64]
        plo_h = np.ascontiguousarray(
            P[rows_lo].reshape(NCHUNK, TPC, KLOW, 128).transpose(0, 2, 1, 3)
        ).reshape(NCHUNK, KLOW, TPC * 128)
        rows_ov = lo[:, None] + KLOW + aovf[None, :].astype(np.int64)
        pov_h = np.ascontiguousarray(
            P[rows_ov].reshape(NCHUNK, TPC, KOVF, 128).transpose(0, 2, 1, 3)
        ).reshape(NCHUNK, KOVF, TPC * 128)

        # overflow one-hot over the last SOVF edges of each tile
        suffix = local[:, TILE - SOVF :]                    # [NTILE, SOVF]
        hov_h = (
            suffix[:, None, :] == (KLOW + aovf)[None, :, None]
        )
        hov_h = np.ascontiguousarray(
            hov_h.reshape(NCHUNK, TPC, KOVF, SOVF).transpose(0, 2, 1, 3)
        ).astype(np.int8).view(np.uint8)
        # 1.0 in e3m4 is 0x30
        hov_h = (hov_h * 0x30).astype(np.uint8).view(E3).reshape(
            NCHUNK, KOVF, TPC * SOVF
        )

        in_maps.append(
            {
                "mv": mv_h,
                "plo": plo_h,
                "pov": pov_h,
                "hov": hov_h,
                "werep": werep_h,
            }
        )
    return in_maps, orders


def kernel(atom_embedding, edge_embedding, src_idx, W, b):
    global LAST_RESULTS
    from concourse.bass_utils import run_bass_kernel_spmd

    nc = _get_module()
    in_maps, orders = _prep_core_inputs(
        atom_embedding, edge_embedding, src_idx, W, b
    )

    kwargs = {}
    if TRACE:
        _install_axon_ntff_shim()
        import concourse.bass_utils as bu

        bu.upload_artifacts = lambda tmpdir: tmpdir  # no bucket in this sandbox
        kwargs = dict(trace=True)

    res = run_bass_kernel_spmd(nc, in_maps, core_ids=list(range(N_CORES)), **kwargs)
    LAST_RESULTS = res

    out = np.empty((N_EDGES, NODE_DIM), np.float32)
    for c in range(N_CORES):
        outt = np.asarray(res.results[c]["outt"])   # [128, EPAD] e3m4
        sorted_out = outt[:, :EPC].T.astype(np.float32)
        out[c * EPC + orders[c]] = sorted_out
    return out


# revision 24
# speedup vs baseline: 1.3553x; 1.2550x over previous
"""Trainium2 Bass kernel for Atom2Bond GNN message passing (forward).

Computation: out[e, :] = relu(concat(atom[src_idx[e]], edge[e]) @ W + b)
  atom_embedding [10000, 128] f32, edge_embedding [640000, 64] f32,
  src_idx [640000] int, W [192, 128] f32, b [128] f32 -> out [640000, 128] f32

Strategy (8 NeuronCores, edges sharded 80000/core, padded to 81920):

  Host-side, per core, edges are SORTED by src_idx. For a 512-edge tile
  whose (sorted) source atoms span [lo, lo+K), the gathered atom matrix
  is piecewise constant in runs, so with the step matrix
      H[a, e] = 1 if e >= start_a else 0        (a = lo..lo+127, local)
  and the first-difference matrix dA[a] = atom[a] - atom[a-1] (dA[lo] =
  atom[lo]), the atom-side contribution telescopes:
      atom[src[e]] = sum_a dA[a] * H[a, e].
  Pre-multiplying by the atom half of W HOST-side, G_t = dA_tile @ Wa,
  the whole gather + atom matmul collapses to ONE on-device matmul:
      out_atom[o, e] = sum_a G_t[a, o] * H_t[a, e]  =  (G_t.T @ H_t)
  H_t is built on-chip in one DVE tensor_scalar(is_ge) op from a
  constant iota row and a per-tile per-partition "starts" vector.

  Per 512-edge tile: 1 DVE tensor_scalar op (H) and 1 atom matmul
  (K=128). The K=64 edge matmuls run pairwise-concurrent on disjoint
  64-row groups of the PE array (row tiling) against host-pre-paired,
  pre-transposed edge features. Four tiles share one 4-bank PSUM
  supertile; its fused bias+ReLU epilogue alternates between the
  scalar engine (activation) and the vector engine (tensor_scalar
  add+max), and each supertile is drained to HBM immediately on the
  otherwise-idle GpSimd SWDGE queue. No gather, no on-chip
  transposes. fp16 on-chip (exact 0/1 for H, ~2^-11 rounding for
  data), fp32 PSUM accumulation. Output is written transposed in
  sorted-edge order; the host un-transposes and un-sorts.

  Measured on 8 NeuronCores: ~107-110 us HW exec (warm chip state;
  ~119-123 us when the PE array is thermally throttled) for the full
  640k-edge problem. Per-core HBM traffic is ~36 MB with the 16 SDMA
  engines ~91 us busy and no mid-stream DMA gaps — i.e. the kernel
  runs at the per-core memory roofline. rel err ~1.4e-3 vs the f32
  reference.
"""

import numpy as np
import ml_dtypes

FP16 = np.float16
E3 = ml_dtypes.float8_e3m4

N_NODES = 10000
N_EDGES = 640000
NODE_DIM = 128
EDGE_DIM = 64
N_CORES = 8

EPC = N_EDGES // N_CORES          # 80000 edges per core
TILE = 512                        # edges per matmul tile
CHUNK = 8192                      # edges per pipeline chunk (16 tiles)
TPC = CHUNK // TILE               # 16 tiles per chunk
EPAD = 81920                      # EPC padded to a multiple of CHUNK
NCHUNK = EPAD // CHUNK            # 10
NTILE = EPAD // TILE              # 160 tiles per core
PAD_IDX = N_NODES + 127           # pad edges point past real atoms (zeros)
KROWS = 128                       # atom rows per tile (max span must be < KROWS)

TRACE = False                     # set True from test.py for NTFF profiling
LAST_RESULTS = None               # BassKernelResults of last run

_NC = None                        # cached compiled Bacc module


def _build_module():
    from contextlib import ExitStack

    import concourse.bacc as bacc
    import concourse.mybir as mybir
    import concourse.tile as tile

    nc = bacc.Bacc("TRN2", target_bir_lowering=False, debug=False)

    # Per-chunk-major host layouts so every chunk DMA is fully contiguous.
    gt = nc.dram_tensor(
        "gt", [NCHUNK, KROWS, TPC * 128], mybir.dt.float16, kind="ExternalInput"
    )
    starts = nc.dram_tensor(
        "starts", [NCHUNK, KROWS, TPC], mybir.dt.float32, kind="ExternalInput"
    )
    edget = nc.dram_tensor(
        "edget", [2 * EDGE_DIM, EPAD // 2], mybir.dt.float8e3, kind="ExternalInput"
    )
    we = nc.dram_tensor("we", [2 * EDGE_DIM, 128], mybir.dt.float16, kind="ExternalInput")
    bias = nc.dram_tensor("bias", [128, 1], mybir.dt.float32, kind="ExternalInput")
    iota = nc.dram_tensor("iota", [128, TILE], mybir.dt.float16, kind="ExternalInput")
    outt = nc.dram_tensor("outt", [128, EPAD], mybir.dt.float16, kind="ExternalOutput")

    with tile.TileContext(nc) as tc, ExitStack() as ctx:
        singles = ctx.enter_context(tc.tile_pool(name="singles", bufs=1))
        gtp = ctx.enter_context(tc.tile_pool(name="gtp", bufs=4))
        stp = ctx.enter_context(tc.tile_pool(name="stp", bufs=3))
        edgep = ctx.enter_context(tc.tile_pool(name="edgep", bufs=4))
        outp = ctx.enter_context(tc.tile_pool(name="outp", bufs=3))
        hp = ctx.enter_context(tc.tile_pool(name="hp", bufs=12))
        psump = ctx.enter_context(tc.tile_pool(name="psum", bufs=2, space="PSUM"))

        # singles ride the scalar/gpsimd DMA queues so the sync queue's
        # first instructions are chunk-0's big loads (their ~2.5us
        # completion latencies overlap instead of serializing)
        iota_sb = singles.tile([128, TILE], mybir.dt.float16)
        nc.scalar.dma_start(out=iota_sb[:], in_=iota[:])
        we_sb = singles.tile([2 * EDGE_DIM, 128], mybir.dt.float16)
        nc.scalar.dma_start(out=we_sb[:], in_=we[:])
        b_sb = singles.tile([128, 1], mybir.dt.float32)
        nc.gpsimd.dma_start(out=b_sb[:], in_=bias[:])

        # ~4us of dummy matmuls during the chunk-0 load window primes the
        # PE HAM clock gate to 8/8 before real work arrives (results unused)
        warm = psump.tile([128, 4 * TILE], mybir.dt.float32, tag="ps")
        for _ in range(10):
            nc.tensor.matmul(
                warm[:, 0:TILE], iota_sb[:, 0:128], iota_sb[:], start=True, stop=True
            )

        for c in range(NCHUNK):
            gt_sb = gtp.tile([KROWS, TPC, 128], mybir.dt.float16)
            nc.sync.dma_start(
                out=gt_sb[:], in_=gt[c].rearrange("a (t f) -> a t f", t=TPC)
            )
            st_sb = stp.tile([KROWS, TPC], mybir.dt.float32)
            nc.gpsimd.dma_start(out=st_sb[:], in_=starts[c])
            edge_sb = edgep.tile([2 * EDGE_DIM, CHUNK // 2], mybir.dt.float8e3)
            nc.sync.dma_start(
                out=edge_sb[:],
                in_=edget[:, c * (CHUNK // 2) : (c + 1) * (CHUNK // 2)],
            )

            out_sb = outp.tile([128, CHUNK], mybir.dt.float16)
            # Four 512-edge tiles share one 4-bank PSUM supertile; the
            # fused bias+ReLU epilogue runs once per supertile, mostly on
            # the scalar engine with 1-in-6 on the vector engine. H builds
            # on the vector engine from the constant iota row.
            for jj in range(TPC // 4):
                ps = psump.tile([128, 4 * TILE], mybir.dt.float32)
                # K=64 edge matmuls first: they depend only on the long-
                # prefetched edge chunk, so the PE starts each supertile
                # without waiting on the DVE's H builds. Pairs run
                # concurrently on disjoint 64-row groups (row tiling).
                for pp in range(2):
                    se = slice((2 * jj + pp) * TILE, (2 * jj + pp + 1) * TILE)
                    nc.tensor.matmul(
                        ps[:, 2 * pp * TILE : (2 * pp + 1) * TILE],
                        we_sb[0:EDGE_DIM, :],
                        edge_sb[0:EDGE_DIM, se],
                        start=True,
                        stop=False,
                        tile_position=(0, 0),
                    )
                    nc.tensor.matmul(
                        ps[:, (2 * pp + 1) * TILE : (2 * pp + 2) * TILE],
                        we_sb[EDGE_DIM : 2 * EDGE_DIM, :],
                        edge_sb[EDGE_DIM : 2 * EDGE_DIM, se],
                        start=True,
                        stop=False,
                        tile_position=(64, 0),
                    )
                for k in range(4):
                    j = 4 * jj + k
                    h_sb = hp.tile([KROWS, TILE], mybir.dt.float16)
                    nc.vector.tensor_scalar(
                        h_sb[:],
                        iota_sb[:KROWS, :],
                        st_sb[:, j : j + 1],
                        None,
                        mybir.AluOpType.is_ge,
                    )
                    pshalf = ps[:, k * TILE : (k + 1) * TILE]
                    nc.tensor.matmul(
                        pshalf, gt_sb[:, j, :], h_sb[:], start=False, stop=True
                    )
                ss = slice(4 * jj * TILE, (4 * jj + 4) * TILE)
                stile = c * (TPC // 4) + jj
                if stile % 6 != 5:
                    nc.scalar.activation(
                        out_sb[:, ss],
                        ps[:],
                        mybir.ActivationFunctionType.Relu,
                        bias=b_sb[:],
                    )
                else:
                    nc.vector.tensor_scalar(
                        out_sb[:, ss],
                        ps[:],
                        b_sb[:],
                        0.0,
                        mybir.AluOpType.add,
                        mybir.AluOpType.max,
                    )
                # drain each supertile as soon as its epilogue lands, on the
                # otherwise-idle SWDGE queue
                nc.gpsimd.dma_start(
                    out=outt[:, c * CHUNK + 4 * jj * TILE : c * CHUNK + (4 * jj + 4) * TILE],
                    in_=out_sb[:, ss],
                )

    nc.compile()
    return nc


def _get_module():
    global _NC
    if _NC is None:
        _NC = _build_module()
    return _NC


def _install_axon_ntff_shim():
    """Register the NTFF profile hook that run_bass_kernel_spmd(trace=True)
    expects under axon; the agent image lacks antenv.axon_hooks."""
    import sys
    import types

    if "antenv.axon_hooks" in sys.modules:
        return
    try:
        from trn_agent_boot.trn_boot import _ntff_profile_via_ctypes

        hook = _ntff_profile_via_ctypes("/opt/axon/libaxon_pjrt.so")
    except Exception:
        hook = None
    mod = types.ModuleType("antenv.axon_hooks")
    mod.get_axon_ntff_profile_hook = lambda: hook
    mod.set_axon_ntff_profile_hook = lambda h: None
    sys.modules["antenv.axon_hooks"] = mod


def _prep_core_inputs(atom_embedding, edge_embedding, src_idx, W, b):
    """Host-side shard + sort + layout prep. Returns (in_maps, orders)."""
    atom_embedding = np.asarray(atom_embedding, dtype=np.float32)
    edge_embedding = np.asarray(edge_embedding, dtype=np.float32)
    src_idx = np.asarray(src_idx).astype(np.int64)
    W = np.asarray(W, dtype=np.float32)
    b = np.asarray(b, dtype=np.float32)

    # P[i] = atom_pad[i] @ Wa ; padded so any tile row slice is in range.
    n_pad = PAD_IDX + 1 + 128
    atom_pad = np.zeros((n_pad, NODE_DIM), np.float32)
    atom_pad[:N_NODES] = atom_embedding
    P = atom_pad @ W[:NODE_DIM]                    # [n_pad, 128] f32
    Pd = np.empty_like(P)                          # Pd[i] = P[i] - P[i-1]
    Pd[0] = P[0]
    Pd[1:] = P[1:] - P[:-1]

    we_h = np.ascontiguousarray(
        np.concatenate([W[NODE_DIM:], W[NODE_DIM:]], axis=0)
    ).astype(FP16)
    b_h = np.ascontiguousarray(b.reshape(NODE_DIM, 1))
    iota_h = np.broadcast_to(
        np.arange(TILE, dtype=np.float32).astype(FP16), (128, TILE)
    ).copy()

    a128 = np.arange(128)
    in_maps = []
    orders = []
    for c in range(N_CORES):
        e0 = c * EPC
        idx_core = src_idx[e0 : e0 + EPC]
        order = np.argsort(idx_core, kind="stable")
        orders.append(order)
        sorted_idx = idx_core[order]
        # pad edges reuse the core's max atom id: keeps sort order and
        # keeps the last tile's atom span tight (outputs are discarded)
        sidx = np.full(EPAD, sorted_idx[-1], np.int64)
        sidx[:EPC] = sorted_idx

        tiles = sidx.reshape(NTILE, TILE)
        lo = tiles[:, 0]                            # [NTILE]
        span = tiles[:, -1] - lo
        assert span.max() < KROWS, (
            f"tile atom span {span.max()} >= {KROWS}; sorted-tile assumption broken"
        )

        # G[t, k] = P[lo_t + k] - P[lo_t + k - 1], with G[t, 0] = P[lo_t]
        rows = lo[:, None] + a128[None, :KROWS]     # [NTILE, KROWS]
        G = Pd[rows]                                # [NTILE, KROWS, 128] f32
        G[:, 0] = P[lo]
        # chunk-major, atom-partition-major layout: [NCHUNK, Ka, TPC, 128f]
        gt_h = np.ascontiguousarray(
            G.reshape(NCHUNK, TPC, KROWS, 128).transpose(0, 2, 1, 3)
        ).astype(FP16).reshape(NCHUNK, KROWS, TPC * 128)

        # starts[t, k] = first within-tile position with idx >= lo_t + k
        st = np.empty((NTILE, KROWS), np.int32)
        for t in range(NTILE):
            st[t] = np.searchsorted(tiles[t], lo[t] + a128[:KROWS], side="left")
        starts_h = np.ascontiguousarray(
            st.reshape(NCHUNK, TPC, KROWS).transpose(0, 2, 1)
        ).astype(np.float32)

        edge_sorted = np.zeros((EPAD, EDGE_DIM), np.float32)
        edge_sorted[:EPC] = edge_embedding[e0 : e0 + EPC][order]
        # pair layout: rows 0-63 = even tiles' features, 64-127 = odd tiles'
        edget_h = np.ascontiguousarray(
            edge_sorted.reshape(NTILE // 2, 2, TILE, EDGE_DIM).transpose(1, 3, 0, 2)
        ).reshape(2 * EDGE_DIM, EPAD // 2).astype(E3)

        in_maps.append(
            {
                "gt": gt_h,
                "starts": starts_h,
                "edget": edget_h,
                "we": we_h,
                "bias": b_h,
                "iota": iota_h,
            }
        )
    return in_maps, orders


def kernel(atom_embedding, edge_embedding, src_idx, W, b):
    global LAST_RESULTS
    from concourse.bass_utils import run_bass_kernel_spmd

    nc = _get_module()
    in_maps, orders = _prep_core_inputs(
        atom_embedding, edge_embedding, src_idx, W, b
    )

    kwargs = {}
    if TRACE:
        _install_axon_ntff_shim()
        import concourse.bass_utils as bu

        bu.upload_artifacts = lambda tmpdir: tmpdir  # no bucket in this sandbox
        kwargs = dict(trace=True)

    res = run_bass_kernel_spmd(nc, in_maps, core_ids=list(range(N_CORES)), **kwargs)
    LAST_RESULTS = res

    out = np.empty((N_EDGES, NODE_DIM), np.float32)
    for c in range(N_CORES):
        outt = np.asarray(res.results[c]["outt"])   # [128, EPAD] fp16
        sorted_out = outt[:, :EPC].T.astype(np.float32)
        out[c * EPC + orders[c]] = sorted_out
    return out

